# revision 11
# baseline (speedup 1.0000x reference)
"""HMM posterior kernel for Trainium2 (8 NeuronCores, SPMD data-parallel over batch).

Math: in the reference,
    ln_fs + ln_bs = 2*ln_pi + ln_emis[:,T-1,:] + total + (T-1)*ln_diag
(the cumsum terms cancel), so the pre-normalization log_gamma is independent
of t and the output is a [B, K] tensor broadcast over T.  With
    S1[b] = sum_t x, S2[b] = sum_t x^2, xl[b] = x[T-1],
    u = S2 + xl^2, v = S1 + xl, P' = exp(-2*ls),
the pre-norm value is rank-2 in the batch:
    g[b,k] = Q[k]*v[b] + P'[k]*(-u[b]/2) + R[k]
    Q = P'*mu
    R = 2*ln_pi + (T-1)*ln_diag - (T+1)*ls - 0.5*(T+1)*P'*mu^2
(a k-independent constant cancels in the normalization), and
out[b,t,:] = g[b,:] - logsumexp_k g[b,:] for every t.

P', Q, R are tiny [K] host-computed fp16 rows.  Device dataflow (per core,
4 batch rows):
  - obvs loads CONTIGUOUS as [64, 128] (partition q = b*16 + s, 512 B
    chunks - cheap descriptors), DVE writes -x^2/2 next to x in one fused
    scalar_tensor_tensor, one fused reduce gives per-partition partials.
  - Two tiny PE matmuls with the partials as *weights* and a block
    selector as rhs land [2, 4] = (v | -u/2) directly in transposed form;
    an ACT cast plus a memset ones-row make the [3, 4] fp16 lhsT.
  - ONE PE matmul [3,4]x[3,K] forms g for all 4 batch rows in f32 PSUM.
  - logsumexp per partition row; the normalize is one fused DVE
    tensor_scalar (psG + negm) - nls with an fp16 cast.
  - Four [128, K] PSUM broadcasts (PE selector matmuls); each PSUM ->
    SBUF fp16 cast is split in half across DVE and ACT (the SBUF tile
    packs TWO fp16 copies of the row per partition -> 2 KB DMA lines).
  - The four 2 MiB output writes alternate between the two hardware DGE
    rings (sync / scalar queues); gpsimd's software path is avoided.
The host upcasts the fp16 output to f32 (scale-relative error ~1e-3 vs
the 2e-2 gate).  The kernel is output-write bound (memory regime,
~373 GB/s sustained).
"""

import numpy as np

B, T, K = 32, 2048, 512
NCORES = 8
BS = B // NCORES  # 4 batch rows per core
WS = 128          # t = s*WS + w layout: partition q = b*16 + s
NQ = BS * 16      # 64 partitions of obvs data
RJ2 = T // 256    # 8 stride-0 repeats of a [128, 2K] fp16 tile per batch row

_BUILT = {}


def _const_blk() -> np.ndarray:
    # [64, 8] f32: col b in 0..3 = 1.0 for partitions q in block b (sums),
    # col 4+b = 1.0 at q = 16b+15 (selects the t=T-1 chunk's partition).
    m = np.zeros((NQ, 8), dtype=np.float32)
    for b in range(BS):
        m[b * 16 : (b + 1) * 16, b] = 1.0
        m[b * 16 + 15, 4 + b] = 1.0
    return m


def _build_nc(split_waits=True):
    key = ("nc", split_waits)
    if key in _BUILT:
        return _BUILT[key]

    from concourse import bass, tile
    import concourse.mybir as mybir

    f32 = mybir.dt.float32
    f16 = mybir.dt.float16
    AF = mybir.ActivationFunctionType
    ALU = mybir.AluOpType
    X = mybir.AxisListType.X

    nc = bass.Bass()
    obvs = nc.declare_dram_parameter("obvs", [BS, T], f32, isOutput=False)
    c_blk = nc.declare_dram_parameter("c_blk", [NQ, 8], f32, isOutput=False)
    c_pack = nc.declare_dram_parameter("c_pack", [BS, 2 * K], f16, isOutput=False)
    out = nc.declare_dram_parameter("out", [BS, T, K], f16, isOutput=True)

    with tile.TileContext(nc) as tc:
        with (
            tc.tile_pool(name="sbuf", bufs=1) as pool,
            tc.tile_pool(name="psum", bufs=1, space="PSUM") as psum,
        ):
            # ---- loads: sync: obvs (contiguous 512B chunks); gpsimd:
            # both const packs (tiny) + the ones-row memset.
            cm4 = pool.tile([NQ, 2, WS], f32)
            nc.sync.dma_start(
                out=cm4[:, 0], in_=obvs[:].rearrange("b (s w) -> (b s) w", w=WS)
            )
            blk = pool.tile([NQ, 8], f32)
            nc.gpsimd.dma_start(out=blk[:], in_=c_blk[:])
            pk = pool.tile([BS, 2 * K], f16)
            nc.gpsimd.dma_start(out=pk[:], in_=c_pack[:])
            selB = pk[:, 0 : 4 * 128]
            prm3 = pk[0:3, K : 2 * K]  # rows: Q | P' | R
            uvT3 = pool.tile([3, BS], f16)
            # ones-row: memset the whole tile, the uv cast overwrites 0:2
            nc.gpsimd.memset(uvT3[:], 1.0)

            # ---- obvs stats: -x^2/2 alongside x, one fused reduce ----
            nc.vector.scalar_tensor_tensor(
                out=cm4[:, 1], in0=cm4[:, 0], scalar=-0.5, in1=cm4[:, 0],
                op0=ALU.mult, op1=ALU.mult,
            )
            red = pool.tile([NQ, 2], f32)
            nc.vector.reduce_sum(red[:].unsqueeze(2), cm4[:], axis=X)

            # ---- PE: ps_uvT[0,:] = v = S1+xl ; ps_uvT[1,:] = -u/2 ----
            ps_uvT = psum.tile([2, BS], f32)
            nc.tensor.matmul(
                ps_uvT[:], lhsT=red[:], rhs=blk[:, 0:BS], start=True, stop=False
            )
            nc.tensor.matmul(
                ps_uvT[:],
                lhsT=cm4[:, :, WS - 1],
                rhs=blk[:, 4 : 4 + BS],
                start=False,
                stop=True,
            )
            nc.scalar.copy(uvT3[0:2, :], ps_uvT[:])

            # ---- ONE matmul forms g for all 4 rows: [3,4]x[3,K] ----
            psG = psum.tile([BS, K], f32)
            nc.tensor.matmul(
                psG[:], lhsT=uvT3[:], rhs=prm3, start=True, stop=True
            )

            # ---- logsumexp + fused normalize/cast ----
            negm = pool.tile([BS, 1], f32)
            nc.vector.reduce_max(negm[:], psG[:], axis=X, negate=True)
            e4 = pool.tile([BS, K], f32)
            ssum = pool.tile([BS, 1], f32)
            nc.scalar.activation(
                e4[:], psG[:], AF.Exp, bias=negm[:], accum_out=ssum[:]
            )
            nls = pool.tile([BS, 1], f32)
            nc.scalar.activation(nls[:], ssum[:], AF.Ln)
            gn = pool.tile([BS, K], f16)
            nc.vector.tensor_scalar(
                out=gn[:], in0=psG[:], scalar1=negm[:], scalar2=nls[:],
                op0=ALU.add, op1=ALU.subtract,
            )

            # ---- broadcast + half-casts + writes on the two HW rings ----
            psBs = []
            for b in range(BS):
                psB = psum.tile([128, K], f32, tag=f"psb{b}", name=f"psb{b}")
                nc.tensor.matmul(
                    psB[:],
                    lhsT=selB[:, b * 128 : (b + 1) * 128],
                    rhs=gn[:],
                    start=True,
                    stop=True,
                )
                psBs.append(psB)
            # Each bt partition holds the SAME row (broadcast), so any
            # partition can source any t-range.  DMA engine = E64 + p//8,
            # and E79 (partitions 120-127) is a chronically slow straggler
            # (19-23 B/ns vs 25): give it 24 of its 32 packets (DMA B,
            # j=3) and route the remaining 32 t-rows through a donor
            # engine that rotates with b (DMA C, src partitions 8b..8b+8).
            dma_eng = [nc.sync, nc.scalar, nc.sync, nc.scalar]
            UC = 4            # fp16 row copies per partition -> 4 KB lines
            for b in range(BS):
                bt = pool.tile([128, UC, K], f16, tag=f"bt{b}", name=f"bt{b}")
                nc.vector.tensor_copy(
                    bt[:, 0 : UC // 2, :],
                    psBs[b][:].unsqueeze(1).broadcast_to([128, UC // 2, K]),
                )
                nc.scalar.copy(
                    bt[:, UC // 2 : UC, :],
                    psBs[b][:].unsqueeze(1).broadcast_to([128, UC // 2, K]),
                )
                row = bt[:].rearrange("p u k -> p (u k)")
                ob = out[b]
                dma_eng[b].dma_start(
                    out=ob[0 : 120 * 16].rearrange(
                        "(p j u) k -> p j (u k)", p=120, j=4, u=UC
                    ),
                    in_=row[0:120].unsqueeze(1).broadcast_to([120, 4, UC * K]),
                )
                dma_eng[b].dma_start(
                    out=ob[1920:2016].rearrange(
                        "(p j u) k -> p j (u k)", p=8, j=3, u=UC
                    ),
                    in_=row[120:128].unsqueeze(1).broadcast_to([8, 3, UC * K]),
                )
                dma_eng[b].dma_start(
                    out=ob[2016:2048].rearrange(
                        "(p j u) k -> p j (u k)", p=8, j=1, u=UC
                    ),
                    in_=row[8 * b : 8 * b + 8]
                    .unsqueeze(1)
                    .broadcast_to([8, 1, UC * K]),
                )

    if split_waits:
        _split_multi_waits(nc, mybir)
    _BUILT[key] = nc
    return nc


def _split_multi_waits(nc, mybir):
    """This walrus build allows at most ONE sync wait per instruction.  Split
    any instruction with N>1 waits into N-1 single-wait NoOps on the same
    engine (executed immediately before it by the same sequencer) plus the
    original instruction carrying the final wait."""
    for fn in nc.m.functions:
        for blk in fn.blocks:
            new_insts = []
            for inst in blk.instructions:
                si = inst.sync_info
                if si is not None and len(si.on_wait) > 1:
                    waits = list(si.on_wait)
                    for i, w in enumerate(waits[:-1]):
                        new_insts.append(
                            mybir.InstNoOp(
                                name=f"{inst.name}-sw{i}",
                                engine=inst.engine,
                                sync_info=mybir.SyncInfo(
                                    on_wait=[w], on_update=[]
                                ),
                                bass_nofuse=True,
                            )
                        )
                    inst.sync_info = mybir.SyncInfo(
                        on_wait=[waits[-1]], on_update=list(si.on_update)
                    )
                new_insts.append(inst)
            blk.instructions = new_insts


def _host_params(inputs):
    mu_f = np.asarray(inputs["mu"], dtype=np.float32)
    ls_f = np.asarray(inputs["log_sigma"], dtype=np.float32)
    pi_f = np.asarray(inputs["ln_pi"], dtype=np.float32)
    di_f = np.asarray(inputs["ln_diag"], dtype=np.float32)
    Pp = np.exp(-2.0 * ls_f)
    Q = Pp * mu_f
    R = (
        2.0 * pi_f
        + (T - 1.0) * di_f
        - (T + 1.0) * ls_f
        - 0.5 * (T + 1.0) * Pp * mu_f * mu_f
    )
    pk = np.zeros((BS, 2 * K), dtype=np.float32)
    for b in range(BS):
        pk[b, b * 128 : (b + 1) * 128] = 1.0  # selB
    pk[0, K : 2 * K] = Q
    pk[1, K : 2 * K] = Pp
    pk[2, K : 2 * K] = R
    return {
        "c_pack": np.ascontiguousarray(pk.astype(np.float16)),
        "c_blk": _const_blk(),
    }


def _run(inputs, trace=False, trace_kwargs=None):
    from concourse.bass_utils import run_bass_kernel_spmd

    nc = _build_nc()
    obvs = np.ascontiguousarray(np.asarray(inputs["obvs"], dtype=np.float32))
    params = _host_params(inputs)
    in_maps = [
        {"obvs": obvs[c * BS : (c + 1) * BS], **params} for c in range(NCORES)
    ]
    kw = {}
    if trace:
        kw["trace"] = True
        if trace_kwargs:
            kw["trace_kwargs"] = trace_kwargs
    res = run_bass_kernel_spmd(nc, in_maps, list(range(NCORES)), **kw)
    full = np.empty((B, T, K), dtype=np.float32)
    for c in range(NCORES):
        full[c * BS : (c + 1) * BS] = np.asarray(
            res.results[c]["out"], dtype=np.float32
        )
    return full, res


def kernel(**inputs) -> np.ndarray:
    full, _ = _run(inputs, trace=False)
    return full


# revision 12
# speedup vs baseline: 1.1479x; 1.1479x over previous
"""HMM posterior kernel for Trainium2 (8 NeuronCores, SPMD data-parallel over batch).

Math: in the reference,
    ln_fs + ln_bs = 2*ln_pi + ln_emis[:,T-1,:] + total + (T-1)*ln_diag
(the cumsum terms cancel), so the pre-normalization log_gamma is independent
of t and the output is a [B, K] tensor broadcast over T.  With
    S1[b] = sum_t x, S2[b] = sum_t x^2, xl[b] = x[T-1],
    u = S2 + xl^2, v = S1 + xl, P' = exp(-2*ls),
the pre-norm value is rank-2 in the batch:
    g[b,k] = Q[k]*v[b] + P'[k]*(-u[b]/2) + R[k]
    Q = P'*mu
    R = 2*ln_pi + (T-1)*ln_diag - (T+1)*ls - 0.5*(T+1)*P'*mu^2
(a k-independent constant cancels in the normalization), and
out[b,t,:] = g[b,:] - logsumexp_k g[b,:] for every t.

P', Q, R are tiny [K] host-computed fp16 rows.  Device dataflow (per core,
4 batch rows):
  - obvs loads CONTIGUOUS as [64, 128] (partition q = b*16 + s, 512 B
    chunks - cheap descriptors), DVE writes -x^2/2 next to x in one fused
    scalar_tensor_tensor, one fused reduce gives per-partition partials.
  - Two tiny PE matmuls with the partials as *weights* and a block
    selector as rhs land [2, 4] = (v | -u/2) directly in transposed form;
    an ACT cast plus a memset ones-row make the [3, 4] fp16 lhsT.
  - ONE PE matmul [3,4]x[3,K] forms g for all 4 batch rows in f32 PSUM.
  - logsumexp per partition row; the normalize is one fused DVE
    tensor_scalar (psG + negm) - nls with an fp16 cast.
  - Four [128, K] PSUM broadcasts (PE selector matmuls); each PSUM ->
    SBUF fp16 cast is split in half across DVE and ACT (the SBUF tile
    packs TWO fp16 copies of the row per partition -> 2 KB DMA lines).
  - The four 2 MiB output writes alternate between the two hardware DGE
    rings (sync / scalar queues); gpsimd's software path is avoided.
The host upcasts the fp16 output to f32 (scale-relative error ~1e-3 vs
the 2e-2 gate).  The kernel is output-write bound (memory regime,
~373 GB/s sustained).
"""

import numpy as np

B, T, K = 32, 2048, 512
NCORES = 8
BS = B // NCORES  # 4 batch rows per core
WS = 128          # t = s*WS + w layout: partition q = b*16 + s
NQ = BS * 16      # 64 partitions of obvs data
RJ2 = T // 256    # 8 stride-0 repeats of a [128, 2K] fp16 tile per batch row

_BUILT = {}


def _const_blk() -> np.ndarray:
    # [64, 8] f32: col b in 0..3 = 1.0 for partitions q in block b (sums),
    # col 4+b = 1.0 at q = 16b+15 (selects the t=T-1 chunk's partition).
    m = np.zeros((NQ, 8), dtype=np.float32)
    for b in range(BS):
        m[b * 16 : (b + 1) * 16, b] = 1.0
        m[b * 16 + 15, 4 + b] = 1.0
    return m


def _build_nc(split_waits=True):
    key = ("nc", split_waits)
    if key in _BUILT:
        return _BUILT[key]

    from concourse import bass, tile
    import concourse.mybir as mybir

    f32 = mybir.dt.float32
    f16 = mybir.dt.float16
    AF = mybir.ActivationFunctionType
    ALU = mybir.AluOpType
    X = mybir.AxisListType.X

    nc = bass.Bass()
    obvs = nc.declare_dram_parameter("obvs", [BS, T], f32, isOutput=False)
    c_blk = nc.declare_dram_parameter("c_blk", [NQ, 8], f32, isOutput=False)
    c_pack = nc.declare_dram_parameter("c_pack", [BS, 2 * K], f16, isOutput=False)
    out = nc.declare_dram_parameter("out", [BS, T, K], f16, isOutput=True)

    with tile.TileContext(nc) as tc:
        with (
            tc.tile_pool(name="sbuf", bufs=1) as pool,
            tc.tile_pool(name="psum", bufs=1, space="PSUM") as psum,
        ):
            # ---- loads: sync: obvs (contiguous 512B chunks); gpsimd:
            # both const packs (tiny) + the ones-row memset.
            cm4 = pool.tile([NQ, 2, WS], f32)
            nc.sync.dma_start(
                out=cm4[:, 0], in_=obvs[:].rearrange("b (s w) -> (b s) w", w=WS)
            )
            blk = pool.tile([NQ, 8], f32)
            nc.gpsimd.dma_start(out=blk[:], in_=c_blk[:])
            pk = pool.tile([BS, 2 * K], f16)
            nc.gpsimd.dma_start(out=pk[:], in_=c_pack[:])
            selB = pk[:, 0 : 4 * 128]
            prm3 = pk[0:3, K : 2 * K]  # rows: Q | P' | R
            uvT3 = pool.tile([3, BS], f16)
            # ones-row: memset the whole tile, the uv cast overwrites 0:2
            nc.gpsimd.memset(uvT3[:], 1.0)

            # ---- obvs stats: -x^2/2 alongside x, one fused reduce ----
            nc.vector.scalar_tensor_tensor(
                out=cm4[:, 1], in0=cm4[:, 0], scalar=-0.5, in1=cm4[:, 0],
                op0=ALU.mult, op1=ALU.mult,
            )
            red = pool.tile([NQ, 2], f32)
            nc.vector.reduce_sum(red[:].unsqueeze(2), cm4[:], axis=X)

            # ---- PE: ps_uvT[0,:] = v = S1+xl ; ps_uvT[1,:] = -u/2 ----
            ps_uvT = psum.tile([2, BS], f32)
            nc.tensor.matmul(
                ps_uvT[:], lhsT=red[:], rhs=blk[:, 0:BS], start=True, stop=False
            )
            nc.tensor.matmul(
                ps_uvT[:],
                lhsT=cm4[:, :, WS - 1],
                rhs=blk[:, 4 : 4 + BS],
                start=False,
                stop=True,
            )
            nc.scalar.copy(uvT3[0:2, :], ps_uvT[:])

            # ---- ONE matmul forms g for all 4 rows: [3,4]x[3,K] ----
            psG = psum.tile([BS, K], f32)
            nc.tensor.matmul(
                psG[:], lhsT=uvT3[:], rhs=prm3, start=True, stop=True
            )

            # ---- logsumexp + fused normalize/cast ----
            negm = pool.tile([BS, 1], f32)
            nc.vector.reduce_max(negm[:], psG[:], axis=X, negate=True)
            e4 = pool.tile([BS, K], f32)
            ssum = pool.tile([BS, 1], f32)
            nc.scalar.activation(
                e4[:], psG[:], AF.Exp, bias=negm[:], accum_out=ssum[:]
            )
            nls = pool.tile([BS, 1], f32)
            nc.scalar.activation(nls[:], ssum[:], AF.Ln)
            gn = pool.tile([BS, K], f16)
            nc.vector.tensor_scalar(
                out=gn[:], in0=psG[:], scalar1=negm[:], scalar2=nls[:],
                op0=ALU.add, op1=ALU.subtract,
            )

            # ---- broadcast + half-casts + writes on the two HW rings ----
            psBs = []
            for b in range(BS):
                psB = psum.tile([128, K], f32, tag=f"psb{b}", name=f"psb{b}")
                nc.tensor.matmul(
                    psB[:],
                    lhsT=selB[:, b * 128 : (b + 1) * 128],
                    rhs=gn[:],
                    start=True,
                    stop=True,
                )
                psBs.append(psB)
            dma_eng = [nc.sync, nc.scalar, nc.sync, nc.scalar]
            UC = 8            # fp16 row copies per partition -> 8 KB lines
            RJ = T // (128 * UC)
            for b in range(BS):
                bt = pool.tile([128, UC, K], f16, tag=f"bt{b}", name=f"bt{b}")
                nc.vector.tensor_copy(
                    bt[:, 0 : UC // 2, :],
                    psBs[b][:].unsqueeze(1).broadcast_to([128, UC // 2, K]),
                )
                nc.scalar.copy(
                    bt[:, UC // 2 : UC, :],
                    psBs[b][:].unsqueeze(1).broadcast_to([128, UC // 2, K]),
                )
                dma_eng[b].dma_start(
                    out=out[b].rearrange("(p j u) k -> p j (u k)", j=RJ, u=UC),
                    in_=bt[:].rearrange("p u k -> p (u k)")
                    .unsqueeze(1)
                    .broadcast_to([128, RJ, UC * K]),
                )

    if split_waits:
        _split_multi_waits(nc, mybir)
    _BUILT[key] = nc
    return nc


def _split_multi_waits(nc, mybir):
    """This walrus build allows at most ONE sync wait per instruction.  Split
    any instruction with N>1 waits into N-1 single-wait NoOps on the same
    engine (executed immediately before it by the same sequencer) plus the
    original instruction carrying the final wait."""
    for fn in nc.m.functions:
        for blk in fn.blocks:
            new_insts = []
            for inst in blk.instructions:
                si = inst.sync_info
                if si is not None and len(si.on_wait) > 1:
                    waits = list(si.on_wait)
                    for i, w in enumerate(waits[:-1]):
                        new_insts.append(
                            mybir.InstNoOp(
                                name=f"{inst.name}-sw{i}",
                                engine=inst.engine,
                                sync_info=mybir.SyncInfo(
                                    on_wait=[w], on_update=[]
                                ),
                                bass_nofuse=True,
                            )
                        )
                    inst.sync_info = mybir.SyncInfo(
                        on_wait=[waits[-1]], on_update=list(si.on_update)
                    )
                new_insts.append(inst)
            blk.instructions = new_insts


def _host_params(inputs):
    mu_f = np.asarray(inputs["mu"], dtype=np.float32)
    ls_f = np.asarray(inputs["log_sigma"], dtype=np.float32)
    pi_f = np.asarray(inputs["ln_pi"], dtype=np.float32)
    di_f = np.asarray(inputs["ln_diag"], dtype=np.float32)
    Pp = np.exp(-2.0 * ls_f)
    Q = Pp * mu_f
    R = (
        2.0 * pi_f
        + (T - 1.0) * di_f
        - (T + 1.0) * ls_f
        - 0.5 * (T + 1.0) * Pp * mu_f * mu_f
    )
    pk = np.zeros((BS, 2 * K), dtype=np.float32)
    for b in range(BS):
        pk[b, b * 128 : (b + 1) * 128] = 1.0  # selB
    pk[0, K : 2 * K] = Q
    pk[1, K : 2 * K] = Pp
    pk[2, K : 2 * K] = R
    return {
        "c_pack": np.ascontiguousarray(pk.astype(np.float16)),
        "c_blk": _const_blk(),
    }


def _run(inputs, trace=False, trace_kwargs=None):
    from concourse.bass_utils import run_bass_kernel_spmd

    nc = _build_nc()
    obvs = np.ascontiguousarray(np.asarray(inputs["obvs"], dtype=np.float32))
    params = _host_params(inputs)
    in_maps = [
        {"obvs": obvs[c * BS : (c + 1) * BS], **params} for c in range(NCORES)
    ]
    kw = {}
    if trace:
        kw["trace"] = True
        if trace_kwargs:
            kw["trace_kwargs"] = trace_kwargs
    res = run_bass_kernel_spmd(nc, in_maps, list(range(NCORES)), **kw)
    full = np.empty((B, T, K), dtype=np.float32)
    for c in range(NCORES):
        full[c * BS : (c + 1) * BS] = np.asarray(
            res.results[c]["out"], dtype=np.float32
        )
    return full, res


def kernel(**inputs) -> np.ndarray:
    full, _ = _run(inputs, trace=False)
    return full
